# revision 2
# baseline (speedup 1.0000x reference)
"""CRF log-partition (forward algorithm) kernel for Trainium2, 8 NeuronCores.

Problem: emissions [64, 512, 1, 128], transitions [1, 128, 128],
start/end transitions [1, 128], ragged lengths [64] in 1..512.
Output: log-partition per (batch, conjugate) -> [64, 1] float32.

Strategy
--------
Data-parallel over batch: 8 batches per core. The forward recurrence is
rewritten in the exp domain so each step is one real matmul plus one
elementwise multiply:

    expU_t[j, b] = exp(e'_t[j, b]) * sum_i expT[i, j] * expU_{t-1}[i, b]

where e'_t = e_t - c_t[b] is host-shifted by c_t[b] = logsumexp_j(e_t[b, j])
so the state stays O(1) in fp32 forever (no device renormalization; the
drift of the column sums is a +-1 nat random walk).  True alpha_t =
log(expU_t) + cumsum(c)[t].

Ragged lengths are handled by *extract-at-length* instead of masking: all
512 state snapshots are kept in SBUF (128 x 4096 fp32 = 2 MB), and an
interleaved matmul against exp(end_transitions) reduces each snapshot to
endsum[t, b]; the host picks column t = len[b]-1 and adds the prefix
normalizer.
"""

import numpy as np

B, L, C, N = 64, 512, 1, 128
N_CORES = 8
B_LOCAL = B // N_CORES  # 8
CT = 64                 # timesteps per DMA/exp chunk
N_CHUNKS = L // CT      # 8

_CACHE = {}


def _build_program():
    """Build (once) the Bass program shared by all 8 cores."""
    if "nc" in _CACHE:
        return _CACHE["nc"]

    from contextlib import ExitStack

    import concourse.bass as bass
    import concourse.tile as tile
    from concourse import bacc, mybir

    f32 = mybir.dt.float32
    FB = L * B_LOCAL  # 4096 free columns of the snapshot buffer

    nc = bacc.Bacc(
        "TRN2",
        debug=False,
        enable_asserts=False,
        target_bir_lowering=False,
        num_devices=N_CORES,
    )

    eh_d = nc.dram_tensor("ehat", [N, FB], f32, kind="ExternalInput").ap()
    traw_d = nc.dram_tensor("traw", [N, N], f32, kind="ExternalInput").ap()
    endraw_d = nc.dram_tensor("endraw", [N, 1], f32, kind="ExternalInput").ap()
    out_d = nc.dram_tensor("lnendsum", [1, FB], f32, kind="ExternalOutput").ap()

    with tile.TileContext(nc) as tc:
        with ExitStack() as ctx:
            consts = ctx.enter_context(tc.tile_pool(name="consts", bufs=1))
            snapp = ctx.enter_context(tc.tile_pool(name="snap", bufs=1))
            rawp = ctx.enter_context(tc.tile_pool(name="raw", bufs=3))
            expp = ctx.enter_context(tc.tile_pool(name="expe", bufs=3))
            psum = ctx.enter_context(tc.tile_pool(name="w", bufs=4, space="PSUM"))
            psum_e = ctx.enter_context(
                tc.tile_pool(name="esum", bufs=2, space="PSUM")
            )

            # Constants: exp() of transitions / end_transitions, on device.
            traw_sb = consts.tile([N, N], f32)
            nc.sync.dma_start(traw_sb[:], traw_d)
            expT_sb = consts.tile([N, N], f32)
            nc.scalar.activation(
                expT_sb[:], traw_sb[:], mybir.ActivationFunctionType.Exp
            )
            endraw_sb = consts.tile([N, 1], f32)
            nc.sync.dma_start(endraw_sb[:], endraw_d)
            expEnd_sb = consts.tile([N, 1], f32)
            nc.scalar.activation(
                expEnd_sb[:], endraw_sb[:], mybir.ActivationFunctionType.Exp
            )

            snap = snapp.tile([N, FB], f32)       # all 512 states, 16 KB/part
            lnsum_sb = consts.tile([1, FB], f32)  # ln(endsum) staging

            expe = []  # per-chunk exp(e') tiles
            for k in range(N_CHUNKS):
                raw = rawp.tile([N, CT * B_LOCAL], f32, tag="raw")
                nc.sync.dma_start(raw[:], eh_d[:, bass.ts(k, CT * B_LOCAL)])
                ec = expp.tile([N, CT * B_LOCAL], f32, tag="expe")
                nc.scalar.activation(
                    ec[:], raw[:], mybir.ActivationFunctionType.Exp
                )
                expe.append(ec)

                if k == 0:
                    # Initial state: expU_0 = exp(e'_0) straight into slot 0.
                    nc.vector.tensor_copy(snap[:, 0:B_LOCAL], ec[:, 0:B_LOCAL])

                # The recurrence for this chunk's timesteps.
                t_lo = k * CT
                for t in range(max(t_lo, 1), t_lo + CT):
                    tl = t - t_lo
                    w = psum.tile([N, B_LOCAL], f32, tag="w")
                    nc.tensor.matmul(
                        w[:],
                        lhsT=expT_sb[:],
                        rhs=snap[:, bass.ts(t - 1, B_LOCAL)],
                        start=True,
                        stop=True,
                    )
                    nc.vector.tensor_mul(
                        snap[:, bass.ts(t, B_LOCAL)],
                        w[:],
                        ec[:, bass.ts(tl, B_LOCAL)],
                    )

                # endsum over this chunk's 64 snapshots:
                # [1, 512] = expEnd^T @ snap_chunk, then ln on ScalarE.
                es = psum_e.tile([1, CT * B_LOCAL], f32, tag="esum")
                nc.tensor.matmul(
                    es[:],
                    lhsT=expEnd_sb[:],
                    rhs=snap[:, bass.ts(k, CT * B_LOCAL)],
                    start=True,
                    stop=True,
                )
                nc.scalar.activation(
                    lnsum_sb[:, bass.ts(k, CT * B_LOCAL)],
                    es[:],
                    mybir.ActivationFunctionType.Ln,
                )

            nc.sync.dma_start(out_d, lnsum_sb[:])

    nc.compile()
    _CACHE["nc"] = nc
    return nc


def _host_prep(emissions, transitions, start_transitions, end_transitions):
    """Build per-core input maps. All fp32."""
    e = np.asarray(emissions, np.float32)[:, :, 0, :]        # [B, L, N]
    start = np.asarray(start_transitions, np.float32)[0]     # [N]
    traw = np.ascontiguousarray(np.asarray(transitions, np.float32)[0])  # [i,j]
    endraw = np.ascontiguousarray(
        np.asarray(end_transitions, np.float32)[0][:, None]
    )  # [N,1]

    # e0 folds start transitions; c = per-(b,t) logsumexp normalizer.
    ebias = e.copy()
    ebias[:, 0, :] += start[None, :]
    m = ebias.max(-1)
    c = (m + np.log(np.exp(ebias - m[..., None]).sum(-1))).astype(np.float32)
    ehat = ebias - c[..., None]                              # [B, L, N]
    A = np.cumsum(c.astype(np.float64), axis=1)              # [B, L] prefix

    in_maps = []
    for k in range(N_CORES):
        sl = ehat[k * B_LOCAL : (k + 1) * B_LOCAL]           # [8, L, N]
        # -> [N, L, 8] -> [N, L*8] so each SBUF partition reads contiguously
        ehat_core = np.ascontiguousarray(
            sl.transpose(2, 1, 0).reshape(N, L * B_LOCAL)
        )
        in_maps.append(
            {"ehat": ehat_core, "traw": traw, "endraw": endraw}
        )
    return in_maps, A


def _run_on_cores(in_maps, trace=False):
    from concourse import bass_utils

    nc = _build_program()
    res = bass_utils.run_bass_kernel_spmd(
        nc, in_maps, core_ids=list(range(N_CORES)), trace=trace
    )
    return res


def kernel(emissions, transitions, start_transitions, end_transitions, lengths):
    in_maps, A = _host_prep(
        emissions, transitions, start_transitions, end_transitions
    )
    res = _run_on_cores(in_maps)

    lengths = np.asarray(lengths).astype(np.int64)
    tstar = lengths - 1                                      # [B]
    out = np.empty((B, C), np.float32)
    for k in range(N_CORES):
        lnsum = np.asarray(res.results[k]["lnendsum"]).reshape(L, B_LOCAL)
        for bl in range(B_LOCAL):
            b = k * B_LOCAL + bl
            out[b, 0] = np.float32(lnsum[tstar[b], bl] + A[b, tstar[b]])
    return out


# revision 3
# speedup vs baseline: 1.9665x; 1.9665x over previous
"""CRF log-partition (forward algorithm) kernel for Trainium2, 8 NeuronCores.

Problem: emissions [64, 512, 1, 128], transitions [1, 128, 128],
start/end transitions [1, 128], ragged lengths [64] in 1..512.
Output: log-partition per (batch, conjugate) -> [64, 1] float32.

Strategy
--------
Data-parallel over batch: 8 batches per core. The forward recurrence is
rewritten in the exp domain so each step is one real matmul plus one
elementwise multiply:

    expU_t[j, b] = exp(e'_t[j, b]) * sum_i expT[i, j] * expU_{t-1}[i, b]

where e'_t = e_t - c_t[b] is host-shifted by c_t[b] = logsumexp_j(e_t[b, j])
so the state stays O(1) in fp32 forever (no device renormalization; the
drift of the column sums is a +-1 nat random walk).  True alpha_t =
log(expU_t) + cumsum(c)[t].

Ragged lengths are handled by *extract-at-length* instead of masking: all
512 state snapshots are kept in SBUF (128 x 4096 fp32 = 2 MB), and an
interleaved matmul against exp(end_transitions) reduces each snapshot to
endsum[t, b]; the host picks column t = len[b]-1 and adds the prefix
normalizer.
"""

import numpy as np

B, L, C, N = 64, 512, 1, 128
N_CORES = 8
B_LOCAL = B // N_CORES  # 8
CT = 64                 # timesteps per DMA/exp chunk
N_CHUNKS = L // CT      # 8

_CACHE = {}


def _build_program():
    """Build (once) the Bass program shared by all 8 cores."""
    if "nc" in _CACHE:
        return _CACHE["nc"]

    from contextlib import ExitStack

    import concourse.bass as bass
    import concourse.tile as tile
    from concourse import bacc, mybir

    f32 = mybir.dt.float32
    bf16 = mybir.dt.bfloat16
    FB = L * B_LOCAL  # 4096 free columns of the snapshot buffer

    nc = bacc.Bacc(
        "TRN2",
        debug=False,
        enable_asserts=False,
        target_bir_lowering=False,
        num_devices=N_CORES,
    )

    eh_d = nc.dram_tensor("ehat", [N, FB], f32, kind="ExternalInput").ap()
    traw_d = nc.dram_tensor("traw", [N, N], f32, kind="ExternalInput").ap()
    endraw_d = nc.dram_tensor("endraw", [N, 1], f32, kind="ExternalInput").ap()
    out_d = nc.dram_tensor("lnendsum", [1, FB], f32, kind="ExternalOutput").ap()

    with tile.TileContext(nc) as tc:
        with ExitStack() as ctx:
            consts = ctx.enter_context(tc.tile_pool(name="consts", bufs=1))
            snapp = ctx.enter_context(tc.tile_pool(name="snap", bufs=1))
            rawp = ctx.enter_context(tc.tile_pool(name="raw", bufs=3))
            expp = ctx.enter_context(tc.tile_pool(name="expe", bufs=3))
            psum = ctx.enter_context(tc.tile_pool(name="w", bufs=4, space="PSUM"))
            psum_e = ctx.enter_context(
                tc.tile_pool(name="esum", bufs=2, space="PSUM")
            )

            # Constants: exp() of transitions / end_transitions, on device.
            traw_sb = consts.tile([N, N], f32)
            nc.sync.dma_start(traw_sb[:], traw_d)
            expT_sb = consts.tile([N, N], bf16)
            nc.scalar.activation(
                expT_sb[:], traw_sb[:], mybir.ActivationFunctionType.Exp
            )
            endraw_sb = consts.tile([N, 1], f32)
            nc.sync.dma_start(endraw_sb[:], endraw_d)
            expEnd_sb = consts.tile([N, 1], bf16)
            nc.scalar.activation(
                expEnd_sb[:], endraw_sb[:], mybir.ActivationFunctionType.Exp
            )

            snap = snapp.tile([N, FB], bf16)      # all 512 states, 8 KB/part
            lnsum_sb = consts.tile([1, FB], f32)  # ln(endsum) staging

            expe = []  # per-chunk exp(e') tiles
            for k in range(N_CHUNKS):
                raw = rawp.tile([N, CT * B_LOCAL], f32, tag="raw")
                nc.sync.dma_start(raw[:], eh_d[:, bass.ts(k, CT * B_LOCAL)])
                ec = expp.tile([N, CT * B_LOCAL], f32, tag="expe")
                nc.scalar.activation(
                    ec[:], raw[:], mybir.ActivationFunctionType.Exp
                )
                expe.append(ec)

                if k == 0:
                    # Initial state: expU_0 = exp(e'_0) straight into slot 0.
                    nc.vector.tensor_copy(snap[:, 0:B_LOCAL], ec[:, 0:B_LOCAL])

                # The recurrence for this chunk's timesteps.
                t_lo = k * CT
                for t in range(max(t_lo, 1), t_lo + CT):
                    tl = t - t_lo
                    w = psum.tile([N, B_LOCAL], f32, tag="w")
                    nc.tensor.matmul(
                        w[:],
                        lhsT=expT_sb[:],
                        rhs=snap[:, bass.ts(t - 1, B_LOCAL)],
                        start=True,
                        stop=True,
                    )
                    nc.vector.tensor_mul(
                        snap[:, bass.ts(t, B_LOCAL)],
                        w[:],
                        ec[:, bass.ts(tl, B_LOCAL)],
                    )

                # endsum over this chunk's 64 snapshots:
                # [1, 512] = expEnd^T @ snap_chunk, then ln on ScalarE.
                es = psum_e.tile([1, CT * B_LOCAL], f32, tag="esum")
                nc.tensor.matmul(
                    es[:],
                    lhsT=expEnd_sb[:],
                    rhs=snap[:, bass.ts(k, CT * B_LOCAL)],
                    start=True,
                    stop=True,
                )
                nc.scalar.activation(
                    lnsum_sb[:, bass.ts(k, CT * B_LOCAL)],
                    es[:],
                    mybir.ActivationFunctionType.Ln,
                )

            nc.sync.dma_start(out_d, lnsum_sb[:])

    nc.compile()
    _CACHE["nc"] = nc
    return nc


def _host_prep(emissions, transitions, start_transitions, end_transitions):
    """Build per-core input maps. All fp32."""
    e = np.asarray(emissions, np.float32)[:, :, 0, :]        # [B, L, N]
    start = np.asarray(start_transitions, np.float32)[0]     # [N]
    traw = np.ascontiguousarray(np.asarray(transitions, np.float32)[0])  # [i,j]
    endraw = np.ascontiguousarray(
        np.asarray(end_transitions, np.float32)[0][:, None]
    )  # [N,1]

    # e0 folds start transitions; c = per-(b,t) logsumexp normalizer.
    ebias = e.copy()
    ebias[:, 0, :] += start[None, :]
    m = ebias.max(-1)
    c = (m + np.log(np.exp(ebias - m[..., None]).sum(-1))).astype(np.float32)
    ehat = ebias - c[..., None]                              # [B, L, N]
    A = np.cumsum(c.astype(np.float64), axis=1)              # [B, L] prefix

    in_maps = []
    for k in range(N_CORES):
        sl = ehat[k * B_LOCAL : (k + 1) * B_LOCAL]           # [8, L, N]
        # -> [N, L, 8] -> [N, L*8] so each SBUF partition reads contiguously
        ehat_core = np.ascontiguousarray(
            sl.transpose(2, 1, 0).reshape(N, L * B_LOCAL)
        )
        in_maps.append(
            {"ehat": ehat_core, "traw": traw, "endraw": endraw}
        )
    return in_maps, A


def _run_on_cores(in_maps, trace=False):
    from concourse import bass_utils

    nc = _build_program()
    res = bass_utils.run_bass_kernel_spmd(
        nc, in_maps, core_ids=list(range(N_CORES)), trace=trace
    )
    return res


def kernel(emissions, transitions, start_transitions, end_transitions, lengths):
    in_maps, A = _host_prep(
        emissions, transitions, start_transitions, end_transitions
    )
    res = _run_on_cores(in_maps)

    lengths = np.asarray(lengths).astype(np.int64)
    tstar = lengths - 1                                      # [B]
    out = np.empty((B, C), np.float32)
    for k in range(N_CORES):
        lnsum = np.asarray(res.results[k]["lnendsum"]).reshape(L, B_LOCAL)
        for bl in range(B_LOCAL):
            b = k * B_LOCAL + bl
            out[b, 0] = np.float32(lnsum[tstar[b], bl] + A[b, tstar[b]])
    return out


# revision 5
# speedup vs baseline: 9.8892x; 5.0289x over previous
"""CRF log-partition (forward algorithm) kernel for Trainium2, 8 NeuronCores.

Problem: emissions [64, 512, 1, 128], transitions [1, 128, 128],
start/end transitions [1, 128], ragged lengths [64] in 1..512.
Output: log-partition per (batch, conjugate) -> [64, 1] float32.

Strategy
--------
Data-parallel over batch: 8 batches per core. The forward recurrence is
rewritten in the exp domain so each step is one matmul plus one
elementwise multiply:

    expU_t[j, b] = exp(e'_t[j, b]) * sum_i expT[i, j] * expU_{t-1}[i, b]

where e'_t = e_t - c_t[b] is host-shifted by c_t[b] = logsumexp_j(e_t[b, j])
so the state stays O(1) in fp32 forever (no device renormalization).
True alpha_t = log(expU_t) + cumsum(c)[t].

Ragged lengths are handled by *extract-at-length*: all 512 state
snapshots are kept in SBUF, reduced against exp(end_transitions) by a
tail matmul into endsum[t, b]; the host picks column t = len[b]-1 and
adds the prefix normalizer.

The 511-step serial chain is the latency bottleneck, so it is split into
G=32 segments computed concurrently in lockstep: one matmul with a
strided rhs AP advances all 32 segment-chains at once, and one strided
vector multiply finishes the super-step.  Segments g>=1 start from an
approximate init (the emission softmax 4 steps before the segment) --
the transition matrix is near-rank-1 (T ~ 0.01) so the chain forgets its
init at Birkhoff rate ~0.05/step, and the per-step growth factors
depend only on the state direction, so after burn-in both direction and
scale match the true chain to below bf16 noise (validated < 3e-5 rel).

If transitions are unexpectedly large (slow mixing would break burn-in
convergence), a safe single-chain program is used instead.
"""

import numpy as np

B, L, C, N = 64, 512, 1, 128
N_CORES = 8
BL = B // N_CORES        # 8 batches per core
FB = L * BL              # 4096 = free columns of snapshot/emission buffers

G = 32                   # concurrent segment-chains per core
SEG = L // G             # 16 timesteps per segment
BURN = 4                 # burn-in steps for segment init convergence

_CACHE = {}


def _build_program_seg():
    """Segmented lockstep program: S = BURN + SEG super-steps."""
    if "seg" in _CACHE:
        return _CACHE["seg"]
    nc = _build(seg=True)
    _CACHE["seg"] = nc
    return nc


def _build_program_chain():
    """Fallback: plain 511-step serial chain (chunked DMA)."""
    if "chain" in _CACHE:
        return _CACHE["chain"]
    nc = _build(seg=False)
    _CACHE["chain"] = nc
    return nc


def _build(seg: bool):
    from contextlib import ExitStack

    import concourse.bass as bass
    import concourse.tile as tile
    from concourse import bacc, mybir

    f32 = mybir.dt.float32
    bf16 = mybir.dt.bfloat16
    Exp = mybir.ActivationFunctionType.Exp
    Ln = mybir.ActivationFunctionType.Ln

    nc = bacc.Bacc(
        "TRN2",
        debug=False,
        enable_asserts=False,
        target_bir_lowering=False,
        num_devices=N_CORES,
    )

    eh_d = nc.dram_tensor("ehat", [N, FB], f32, kind="ExternalInput").ap()
    traw_d = nc.dram_tensor("traw", [N, N], f32, kind="ExternalInput").ap()
    endraw_d = nc.dram_tensor("endraw", [N, 1], f32, kind="ExternalInput").ap()
    out_d = nc.dram_tensor("lnendsum", [1, FB], f32, kind="ExternalOutput").ap()

    with tile.TileContext(nc) as tc:
        with ExitStack() as ctx:
            consts = ctx.enter_context(tc.tile_pool(name="consts", bufs=1))
            snapp = ctx.enter_context(tc.tile_pool(name="snap", bufs=1))
            psum = ctx.enter_context(tc.tile_pool(name="w", bufs=2, space="PSUM"))
            psum_e = ctx.enter_context(
                tc.tile_pool(name="esum", bufs=2, space="PSUM")
            )

            traw_sb = consts.tile([N, N], f32)
            nc.sync.dma_start(traw_sb[:], traw_d)
            expT_sb = consts.tile([N, N], bf16)
            nc.scalar.activation(expT_sb[:], traw_sb[:], Exp)
            endraw_sb = consts.tile([N, 1], f32)
            nc.sync.dma_start(endraw_sb[:], endraw_d)
            expEnd_sb = consts.tile([N, 1], bf16)
            nc.scalar.activation(expEnd_sb[:], endraw_sb[:], Exp)

            snap = snapp.tile([N, FB], bf16)
            snap3 = snap[:].rearrange("p (t b) -> p t b", b=BL)
            lnsum_sb = consts.tile([1, FB], f32)

            if seg:
                _emit_seg(nc, tc, ctx, consts, psum, bass, mybir,
                          eh_d, expT_sb, snap, snap3, Exp)
            else:
                _emit_chain(nc, tc, ctx, psum, bass, mybir,
                            eh_d, expT_sb, snap, snap3, Exp)

            # endsum[t, b] = sum_j expEnd[j] * expU_t[j, b]; then ln.
            for k in range(FB // 512):
                es = psum_e.tile([1, 512], f32, tag="esum")
                nc.tensor.matmul(
                    es[:], lhsT=expEnd_sb[:], rhs=snap[:, bass.ts(k, 512)],
                    start=True, stop=True,
                )
                nc.scalar.activation(lnsum_sb[:, bass.ts(k, 512)], es[:], Ln)

            nc.sync.dma_start(out_d, lnsum_sb[:])

    nc.compile()
    return nc


def _emit_seg(nc, tc, ctx, consts, psum, bass, mybir,
              eh_d, expT_sb, snap, snap3, Exp):
    """G segment-chains advanced in lockstep; one MM + one TT per super-step."""
    f32 = mybir.dt.float32
    bf16 = mybir.dt.bfloat16

    rawp = ctx.enter_context(tc.tile_pool(name="raw", bufs=1))
    raw_all = rawp.tile([N, FB], f32)
    expe = consts.tile([N, FB], f32)
    expe3 = expe[:].rearrange("p (t b) -> p t b", b=BL)
    # DMA + exp in 4 slices so they pipeline
    for q in range(4):
        nc.sync.dma_start(raw_all[:, bass.ts(q, FB // 4)],
                          eh_d[:, bass.ts(q, FB // 4)])
        nc.scalar.activation(expe[:, bass.ts(q, FB // 4)],
                             raw_all[:, bass.ts(q, FB // 4)], Exp)

    scratch = consts.tile([N, 2 * G * BL], bf16)
    scratch4 = scratch[:].rearrange("p (h g b) -> p h g b", h=2, b=BL)

    # Initial states: chain 0 = exact exp(e'_0) in snap slot 0; chains
    # g>=1 = emission softmax BURN+1 steps before their segment.
    nc.vector.tensor_copy(snap3[:, 0, :], expe3[:, 0, :])
    nc.vector.memset(scratch[:], 1.0)
    nc.vector.tensor_copy(
        scratch4[:, 1, 1:G, :],
        expe3[:, SEG - BURN - 1 : (G - 1) * SEG : SEG, :],
    )

    S = BURN + SEG
    for s in range(S):
        w = psum.tile([N, G * BL], f32, tag="w")
        if s == 0:
            rhs = scratch4[:, 1, :, :]
        elif s <= BURN:
            rhs = scratch4[:, (s - 1) % 2, :, :]
        else:
            t0 = s - BURN - 1
            rhs = snap3[:, t0 : t0 + (G - 1) * SEG + 1 : SEG, :]
        nc.tensor.matmul(w[:], lhsT=expT_sb[:], rhs=rhs, start=True, stop=True)

        w3 = w[:].rearrange("p (g b) -> p g b", b=BL)
        if s < BURN:
            # burn-in: chains 1..G-1 -> scratch half s%2
            t0 = SEG + s - BURN
            nc.vector.tensor_mul(
                scratch4[:, s % 2, 1:G, :],
                w3[:, 1:G, :],
                expe3[:, t0 : t0 + (G - 2) * SEG + 1 : SEG, :],
            )
        elif s == BURN:
            # chains 1..G-1 write their first real slot; slot 0 is init
            nc.vector.tensor_mul(
                snap3[:, SEG : (G - 1) * SEG + 1 : SEG, :],
                w3[:, 1:G, :],
                expe3[:, SEG : (G - 1) * SEG + 1 : SEG, :],
            )
        else:
            t0 = s - BURN
            sl = slice(t0, t0 + (G - 1) * SEG + 1, SEG)
            nc.vector.tensor_mul(snap3[:, sl, :], w3[:], expe3[:, sl, :])


def _emit_chain(nc, tc, ctx, psum, bass, mybir,
                eh_d, expT_sb, snap, snap3, Exp):
    """Serial 511-step chain (safe fallback for slow-mixing transitions)."""
    f32 = mybir.dt.float32
    CT = 64
    rawp = ctx.enter_context(tc.tile_pool(name="raw", bufs=3))
    expp = ctx.enter_context(tc.tile_pool(name="expe", bufs=3))
    psum_c = ctx.enter_context(tc.tile_pool(name="wc", bufs=4, space="PSUM"))

    for k in range(L // CT):
        raw = rawp.tile([N, CT * BL], f32, tag="raw")
        nc.sync.dma_start(raw[:], eh_d[:, bass.ts(k, CT * BL)])
        ec = expp.tile([N, CT * BL], f32, tag="expe")
        nc.scalar.activation(ec[:], raw[:], Exp)
        if k == 0:
            nc.vector.tensor_copy(snap[:, 0:BL], ec[:, 0:BL])
        t_lo = k * CT
        for t in range(max(t_lo, 1), t_lo + CT):
            tl = t - t_lo
            w = psum_c.tile([N, BL], f32, tag="wc")
            nc.tensor.matmul(
                w[:], lhsT=expT_sb[:], rhs=snap[:, bass.ts(t - 1, BL)],
                start=True, stop=True,
            )
            nc.vector.tensor_mul(
                snap[:, bass.ts(t, BL)], w[:], ec[:, bass.ts(tl, BL)]
            )


def _host_prep(emissions, transitions, start_transitions, end_transitions):
    e = np.asarray(emissions, np.float32)[:, :, 0, :]        # [B, L, N]
    start = np.asarray(start_transitions, np.float32)[0]
    traw = np.ascontiguousarray(np.asarray(transitions, np.float32)[0])
    endraw = np.ascontiguousarray(
        np.asarray(end_transitions, np.float32)[0][:, None]
    )

    ebias = e.copy()
    ebias[:, 0, :] += start[None, :]
    m = ebias.max(-1)
    c = (m + np.log(np.exp(ebias - m[..., None]).sum(-1))).astype(np.float32)
    ehat = ebias - c[..., None]
    A = np.cumsum(c.astype(np.float64), axis=1)              # [B, L]

    in_maps = []
    for k in range(N_CORES):
        sl = ehat[k * BL : (k + 1) * BL]                     # [8, L, N]
        ehat_core = np.ascontiguousarray(
            sl.transpose(2, 1, 0).reshape(N, L * BL)
        )
        in_maps.append({"ehat": ehat_core, "traw": traw, "endraw": endraw})
    return in_maps, A


def _run_on_cores(in_maps, trace=False, seg=True):
    from concourse import bass_utils

    nc = _build_program_seg() if seg else _build_program_chain()
    return bass_utils.run_bass_kernel_spmd(
        nc, in_maps, core_ids=list(range(N_CORES)), trace=trace
    )


def kernel(emissions, transitions, start_transitions, end_transitions, lengths):
    in_maps, A = _host_prep(
        emissions, transitions, start_transitions, end_transitions
    )
    # Burn-in convergence needs fast mixing; true for this problem's
    # T ~ N(0, 0.01^2). Fall back to the exact serial chain otherwise.
    seg_ok = float(np.abs(np.asarray(transitions)).max()) < 0.15
    res = _run_on_cores(in_maps, seg=seg_ok)

    lengths = np.asarray(lengths).astype(np.int64)
    tstar = lengths - 1
    out = np.empty((B, C), np.float32)
    for k in range(N_CORES):
        lnsum = np.asarray(res.results[k]["lnendsum"]).reshape(L, BL)
        for bl in range(BL):
            b = k * BL + bl
            out[b, 0] = np.float32(lnsum[tstar[b], bl] + A[b, tstar[b]])
    return out
